# revision 14
# baseline (speedup 1.0000x reference)
"""AutoDisBucketEncoder Trainium2 kernel (8 NeuronCores, feature-sharded).

Math (per feature f, batch b):
  h = lrelu(x_aug @ w1_aug)            # bias folded via ones column
  h = lrelu(h @ (rw_l + I) + rb_l)     # x3, residual folded into weights
  z = lrelu(h @ w2 + b2)
  e = exp(z * tau)
  out = (e / sum_k e) @ emb

Layout: features sharded 32/core; each core packs 2 features per 128
partitions (block-diagonal weights), streams the full 2048 batch as the
matmul moving dim.  Softmax runs in [k, b] layout; the sum-over-k and its
broadcast back to 128 partitions are done by one ones-block matmul.  The
embedding matmul uses normalized probabilities as the stationary operand so
its PSUM output lands directly in [batch, emb] layout for linear DMA out.
"""

import sys

sys.path.insert(0, "/opt/trn_rl_repo")

import numpy as np
import ml_dtypes
from contextlib import ExitStack

BF16 = ml_dtypes.bfloat16
B, F, D, K, E = 2048, 256, 64, 8, 128
NCORES = 8
FC = F // NCORES          # 32 features per core
NPAIR = FC // 2           # 16
NSTACK = NPAIR // 4       # 4 stacks of 4 pairs
NEG = 0.01                # leaky slope
HB = B // 2               # 1024 batch half-chunk (2 PSUM banks in f32)

# which residual-layer evictions go to the DVE 2-op path (rest: 1 ACT op)
DVE_RES_MOD = 4
DVE_RES_LIM = 1  # idx % MOD < LIM -> DVE path
# out-psum eviction: idx % MOD != 0 -> ACT, else DVE
OUT_EVICT_DVE_MOD = 1

_compiled = None


def _build_bass():
    import concourse.bass as bass  # noqa: F401
    import concourse.mybir as mybir
    import concourse.tile as tile
    from concourse import bacc

    dt = mybir.dt
    AF = mybir.ActivationFunctionType
    ALU = mybir.AluOpType

    nc = bacc.Bacc("TRN2", target_bir_lowering=False, debug=False)

    xp = nc.dram_tensor("xp", [NPAIR, 8, B], dt.bfloat16, kind="ExternalInput").ap()
    w1p = nc.dram_tensor("w1p", [NPAIR, 8, 128], dt.bfloat16, kind="ExternalInput").ap()
    rwp = nc.dram_tensor("rwp", [3, NPAIR, 128, 128], dt.bfloat16, kind="ExternalInput").ap()
    rbp = nc.dram_tensor("rbp", [3, NPAIR, 128], dt.float32, kind="ExternalInput").ap()
    w2p = nc.dram_tensor("w2p", [NPAIR, 128, 16], dt.bfloat16, kind="ExternalInput").ap()
    b2s = nc.dram_tensor("b2s", [NSTACK, 128], dt.float32, kind="ExternalInput").ap()
    taus = nc.dram_tensor("taus", [NSTACK, 128], dt.float32, kind="ExternalInput").ap()
    onesbd = nc.dram_tensor("onesbd", [128, 128], dt.bfloat16, kind="ExternalInput").ap()
    embs = nc.dram_tensor("embs", [NSTACK, 128, 256], dt.bfloat16, kind="ExternalInput").ap()
    out = nc.dram_tensor("out", [B, FC * E], dt.bfloat16, kind="ExternalOutput").ap()

    with tile.TileContext(nc) as tc, ExitStack() as ctx:
        const = ctx.enter_context(tc.tile_pool(name="const", bufs=1))
        xpool = ctx.enter_context(tc.tile_pool(name="xpool", bufs=3))
        hpool = ctx.enter_context(tc.tile_pool(name="hpool", bufs=4))
        tpool = ctx.enter_context(tc.tile_pool(name="tpool", bufs=3))
        epool = ctx.enter_context(tc.tile_pool(name="epool", bufs=2))
        rpool = ctx.enter_context(tc.tile_pool(name="rpool", bufs=2))
        opool = ctx.enter_context(tc.tile_pool(name="opool", bufs=3))
        h_ps = ctx.enter_context(tc.tile_pool(name="h_ps", bufs=2, space="PSUM"))
        m_ps = ctx.enter_context(tc.tile_pool(name="m_ps", bufs=2, space="PSUM"))

        # ---- constants into SBUF ----
        w1_sb = const.tile([8, NPAIR, 128], dt.bfloat16)
        nc.sync.dma_start(out=w1_sb, in_=w1p.rearrange("p k m -> k p m"))
        rw_sb = const.tile([128, 3 * NPAIR, 128], dt.bfloat16)
        nc.sync.dma_start(out=rw_sb, in_=rwp.rearrange("l p k m -> k (l p) m"))
        rb_sb = const.tile([128, 3 * NPAIR], dt.float32)
        nc.sync.dma_start(out=rb_sb, in_=rbp.rearrange("l p k -> k (l p)"))
        w2_sb = const.tile([128, NPAIR, 16], dt.bfloat16)
        nc.sync.dma_start(out=w2_sb, in_=w2p.rearrange("p k m -> k p m"))
        b2_sb = const.tile([128, NSTACK], dt.float32)
        nc.sync.dma_start(out=b2_sb, in_=b2s.rearrange("s k -> k s"))
        tau_sb = const.tile([128, NSTACK], dt.float32)
        nc.sync.dma_start(out=tau_sb, in_=taus.rearrange("s k -> k s"))
        ones_sb = const.tile([128, 128], dt.bfloat16)
        nc.sync.dma_start(out=ones_sb, in_=onesbd)
        emb_sb = const.tile([128, NSTACK, 256], dt.bfloat16)
        nc.sync.dma_start(out=emb_sb, in_=embs.rearrange("s k m -> k s m"))

        # out[b, fc*E] viewed as [quarter, pair, p(128), i(4), e(256)]
        out_r = out.rearrange(
            "(qb i p) (pr e) -> qb pr p i e", qb=4, i=4, p=128, pr=NPAIR
        )

        res_idx = 0
        out_idx = 0
        for s in range(NSTACK):
            pz = [
                m_ps.tile([128, HB], dt.float32, tag="zso", name=f"pz{s}_{c}")
                for c in range(2)
            ]
            for j in range(4):
                p = 4 * s + j
                x_sb = xpool.tile([8, B], dt.bfloat16, tag="x")
                nc.sync.dma_start(out=x_sb, in_=xp[p])
                for c in range(2):
                    ph = h_ps.tile([128, HB], dt.float32, tag="h")
                    for q in range(2):
                        nc.tensor.matmul(
                            ph[:, q * 512 : (q + 1) * 512],
                            w1_sb[:, p, :],
                            x_sb[:, c * HB + q * 512 : c * HB + (q + 1) * 512],
                            start=True,
                            stop=True,
                        )
                    h = hpool.tile([128, HB], dt.bfloat16, tag="h")
                    # bias already in psum (ones column): plain leaky
                    nc.scalar.activation(h, ph, AF.Lrelu, alpha=NEG)
                    for l in range(3):
                        ph2 = h_ps.tile([128, HB], dt.float32, tag="h")
                        wsl = rw_sb[:, l * NPAIR + p, :]
                        for q in range(2):
                            nc.tensor.matmul(
                                ph2[:, q * 512 : (q + 1) * 512],
                                wsl,
                                h[:, q * 512 : (q + 1) * 512],
                                start=True,
                                stop=True,
                            )
                        h2 = hpool.tile([128, HB], dt.bfloat16, tag="h")
                        rb_ap = rb_sb[:, l * NPAIR + p : l * NPAIR + p + 1]
                        if res_idx % DVE_RES_MOD < DVE_RES_LIM:
                            t = tpool.tile([128, HB], dt.bfloat16, tag="rt")
                            nc.vector.tensor_scalar_add(t, ph2, rb_ap)
                            nc.vector.scalar_tensor_tensor(
                                h2, t, NEG, t, ALU.mult, ALU.max
                            )
                        else:
                            nc.scalar.activation(
                                h2, ph2, AF.Lrelu, bias=rb_ap, alpha=NEG
                            )
                        res_idx += 1
                        h = h2
                    for q in range(2):
                        nc.tensor.matmul(
                            pz[c][32 * j : 32 * j + 16, q * 512 : (q + 1) * 512],
                            w2_sb[:, p, :],
                            h[:, q * 512 : (q + 1) * 512],
                            start=True,
                            stop=True,
                            tile_position=(0, 32 * j),
                        )
            # ---- z epilogue: lrelu(+b2), exp(*tau) ----
            e_sb = epool.tile([128, B], dt.bfloat16, tag="e")
            for c in range(2):
                t1 = tpool.tile([128, HB], dt.float32, tag="zt")
                nc.scalar.activation(
                    t1, pz[c], AF.Lrelu, bias=b2_sb[:, s : s + 1], alpha=NEG
                )
                nc.scalar.activation(
                    e_sb[:, c * HB : (c + 1) * HB],
                    t1,
                    AF.Exp,
                    scale=tau_sb[:, s : s + 1],
                )
            # ---- sum over k (with broadcast back to 128 rows), recip, normalize ----
            en_sb = epool.tile([128, B], dt.bfloat16, tag="en")
            for c in range(2):
                ps_sum = m_ps.tile([128, HB], dt.float32, tag="zso")
                for q in range(2):
                    nc.tensor.matmul(
                        ps_sum[:, q * 512 : (q + 1) * 512],
                        ones_sb,
                        e_sb[:, c * HB + q * 512 : c * HB + (q + 1) * 512],
                        start=True,
                        stop=True,
                    )
                rc = rpool.tile([128, HB], dt.bfloat16, tag="rc")
                with nc.allow_low_precision(
                    reason="bf16 reciprocal of softmax denominator; error budgeted"
                ):
                    nc.vector.reciprocal(rc, ps_sum)
                nc.vector.tensor_mul(
                    en_sb[:, c * HB : (c + 1) * HB],
                    e_sb[:, c * HB : (c + 1) * HB],
                    rc,
                )
            # ---- embedding matmuls: p_norm (stationary) x emb -> [b, e] ----
            for j in range(4):
                p = 4 * s + j
                for qb in range(4):
                    po = m_ps.tile([128, 4, 256], dt.float32, tag="zso")
                    for i in range(4):
                        bc2 = qb * 4 + i
                        nc.tensor.matmul(
                            po[:, i, :],
                            en_sb[32 * j : 32 * j + 16, bc2 * 128 : (bc2 + 1) * 128],
                            emb_sb[32 * j : 32 * j + 16, s, :],
                            start=True,
                            stop=True,
                            tile_position=(32 * j, 0),
                        )
                    ost = opool.tile([128, 4, 256], dt.bfloat16, tag="o")
                    if out_idx % OUT_EVICT_DVE_MOD == 0:
                        nc.vector.tensor_copy(ost, po)
                    else:
                        nc.scalar.copy(ost, po)
                    out_idx += 1
                    nc.sync.dma_start(out=out_r[qb, p], in_=ost)

    nc.compile()
    return nc


def _host_pack(inputs):
    """Pack full f32 inputs into per-core bf16 device arrays."""
    x = np.ascontiguousarray(inputs["x"], dtype=np.float32)
    w1 = np.asarray(inputs["w1"], dtype=np.float32)
    b1 = np.asarray(inputs["b1"], dtype=np.float32)
    w2 = np.asarray(inputs["w2"], dtype=np.float32)
    b2 = np.asarray(inputs["b2"], dtype=np.float32)
    tau = np.asarray(inputs["tau"], dtype=np.float32)
    emb = np.asarray(inputs["emb"], dtype=np.float32)
    rws = [np.asarray(inputs[f"rw{l}"], dtype=np.float32) for l in range(3)]
    rbs = [np.asarray(inputs[f"rb{l}"], dtype=np.float32) for l in range(3)]

    eye = np.eye(D, dtype=np.float32)
    # x_aug^T per feature: [F, 4, B]
    xT = np.concatenate([x, np.ones((B, F, 1), np.float32)], axis=2)
    xT = np.ascontiguousarray(xT.transpose(1, 2, 0))  # [F, 4, B]
    w1a = np.concatenate([w1, b1[:, None, :]], axis=1)  # [F, 4, D]

    in_maps = []
    for cidx in range(NCORES):
        f0 = cidx * FC
        m = {}
        xpk = np.zeros((NPAIR, 8, B), BF16)
        w1k = np.zeros((NPAIR, 8, 128), BF16)
        rwk = np.zeros((3, NPAIR, 128, 128), BF16)
        rbk = np.zeros((3, NPAIR, 128), np.float32)
        w2k = np.zeros((NPAIR, 128, 16), BF16)
        b2k = np.zeros((NSTACK, 128), np.float32)
        tauk = np.zeros((NSTACK, 128), np.float32)
        embk = np.zeros((NSTACK, 128, 256), BF16)
        for pr in range(NPAIR):
            fa, fb = f0 + 2 * pr, f0 + 2 * pr + 1
            xpk[pr, 0:4] = xT[fa]
            xpk[pr, 4:8] = xT[fb]
            w1k[pr, 0:4, 0:64] = w1a[fa]
            w1k[pr, 4:8, 64:128] = w1a[fb]
            for l in range(3):
                rwk[l, pr, 0:64, 0:64] = rws[l][fa] + eye
                rwk[l, pr, 64:128, 64:128] = rws[l][fb] + eye
                rbk[l, pr, 0:64] = rbs[l][fa]
                rbk[l, pr, 64:128] = rbs[l][fb]
            w2k[pr, 0:64, 0:8] = w2[fa]
            w2k[pr, 64:128, 8:16] = w2[fb]
            s, jj = pr // 4, pr % 4
            for fi, ff in ((0, fa), (1, fb)):
                b2k[s, 32 * jj + 8 * fi : 32 * jj + 8 * fi + 8] = b2[ff]
                tauk[s, 32 * jj + 8 * fi : 32 * jj + 8 * fi + 8] = tau[ff]
                embk[s, 32 * jj + 8 * fi : 32 * jj + 8 * fi + 8, 128 * fi : 128 * fi + 128] = emb[ff]
        m["xp"] = xpk
        m["w1p"] = w1k
        m["rwp"] = rwk
        m["rbp"] = rbk
        m["w2p"] = w2k
        m["b2s"] = b2k
        m["taus"] = tauk
        m["embs"] = embk
        # sum-over-k stationary with broadcast to all 128 rows; rows of
        # garbage partitions duplicate the pair's first feature so the
        # reciprocal stays finite.
        ob = np.zeros((128, 128), BF16)
        for jj in range(4):
            for g in range(4):  # 4 groups of 8 rows per 32-block
                src = 32 * jj + 8 * min(g, 1)
                for mcol in range(32 * jj + 8 * g, 32 * jj + 8 * g + 8):
                    ob[src : src + 8, mcol] = 1
        m["onesbd"] = ob
        in_maps.append(m)
    return in_maps


def _get_compiled():
    global _compiled
    if _compiled is None:
        _compiled = _build_bass()
    return _compiled


def run_on_hw(in_maps, trace=False):
    from concourse import bass_utils

    nc = _get_compiled()
    res = bass_utils.run_bass_kernel_spmd(
        nc, in_maps, core_ids=list(range(NCORES)), trace=trace
    )
    return res


def kernel(**inputs):
    in_maps = _host_pack(inputs)
    res = run_on_hw(in_maps, trace=False)
    outs = [np.asarray(res.results[c]["out"], dtype=np.float32) for c in range(NCORES)]
    return np.concatenate(outs, axis=1)
